# revision 1
# baseline (speedup 1.0000x reference)
"""TRN2 Bass kernel v2 for nn_Attention: causal MHA, one batch element per core.

Structure (all matmul operands bf16, fp32 PSUM accumulation):
  A:  xT = transpose(x)  (bf16 PE transposes)
  B2: v = xT^T @ W_v (+ ones col per head for softmax denominators)
  S1: per head-pair m: B1(m) (qkT rows m, m+8) interleaved with C(m, j=0)
  S2: per head-pair m: C(m, j=1), with first-half output projection D(qc 0..3)
      interleaved as PE filler; then D(qc 4..7).
Causal dead columns are skipped in scores/exp/AV; only the 128-wide diagonal
blocks get a gpsimd triangle mask. Softmax denominators ride as a 65th row of
the AV matmul; normalization uses a DVE reciprocal + DRAM-bounce partition
broadcast.
"""

import os
import sys

for _p in ("/opt/trn_rl_repo", os.path.expanduser("~/.axon_site/_ro/trn_rl_repo")):
    if os.path.isdir(_p) and _p not in sys.path:
        sys.path.insert(0, _p)

from contextlib import ExitStack

import numpy as np

import concourse.bass as bass
import concourse.tile as tile
from concourse import bacc, mybir
from concourse.masks import make_identity

F32 = mybir.dt.float32
BF = mybir.dt.bfloat16

S = 1024
D = 1024
H = 16
P = 128
NQ = 512
SC = S // P   # 8
DC = D // P   # 8
MQK = 2 * D // P  # 16


def build_kernel(niter=1, psS_bufs=2, po_bufs=2, psb1_bufs=2, psY_bufs=2,
                 e_bufs=6, wq_bufs=6, uniform=False, qk_f32=False, phases="full",
                 ldw_share=False, early_wq=True, pb_bcast=True, d_after=(1, 3, 5, 7),
                 og_stage=False, a_dmaT=True):
    nc = bacc.Bacc("TRN2", target_bir_lowering=False, debug=False, num_devices=8)

    x_ap = nc.dram_tensor("x", [S, D], F32, kind="ExternalInput").ap()
    wqkv_ap = nc.dram_tensor("W_qkv", [D, 3 * D], F32, kind="ExternalInput").ap()
    bqkv_ap = nc.dram_tensor("b_qkv", [3 * D], F32, kind="ExternalInput").ap()
    wout_ap = nc.dram_tensor("W_out", [D, D], F32, kind="ExternalInput").ap()
    bout_ap = nc.dram_tensor("b_out", [D], F32, kind="ExternalOutput" if False else "ExternalInput").ap()
    y_ap = nc.dram_tensor("y", [S, D], F32, kind="ExternalOutput").ap()

    wqkv_r = wqkv_ap.rearrange("(kc p) n -> p kc n", p=P)
    wout_r = wout_ap.rearrange("(kc p) n -> p kc n", p=P)

    QKDT = F32 if qk_f32 else BF

    with tile.TileContext(nc) as tc:
      for _it in range(niter):
        top = ExitStack()
        p_top = top.enter_context(tc.tile_pool(name="p_top", bufs=1))

        if not a_dmaT:
            ident = p_top.tile([P, P], BF)
            make_identity(nc, ident)

        bqkv_sb = p_top.tile([P, 3 * D // P], F32)
        nc.sync.dma_start(bqkv_sb[:], bqkv_ap.rearrange("(m p) -> p m", p=P))

        qkT = p_top.tile([P, MQK, S], QKDT)     # rows of [q;k]^T
        xT = p_top.tile([P, DC, S], BF)         # x^T
        attnT = p_top.tile([P, DC, S], BF)      # attn_out^T (normalized)
        v_sb = p_top.tile([P, SC, H * 65], BF)  # [kv-pos, kv-chunk, head*(64 v + 1 ones)]

        # W_qk chunk prefetch pipeline: allocate + DMA all 16 chunks up front;
        # the wq pool's slot recycling paces the prefetch ~wq_bufs chunks ahead.
        use_early_wq = early_wq and phases not in ("a", "ab")
        wqp0 = tc.alloc_tile_pool(name="wq0", bufs=wq_bufs) if use_early_wq else None
        wq_tiles = {}
        if use_early_wq:
            order = []
            for m in range(H // 2):
                order += [m, 8 + m]
            for mm in order:
                wq = wqp0.tile([P, DC, P], BF, tag="wq", name=f"wq_{mm}")
                nc.gpsimd.dma_start(wq[:], wqkv_r[:, :, mm * P : (mm + 1) * P])
                wq_tiles[mm] = wq

        # W_v for phase B2 (released after)
        wvp = tc.alloc_tile_pool(name="wv", bufs=1)
        wv = wvp.tile([P, DC, D], BF)
        nc.gpsimd.dma_start(wv[:], wqkv_r[:, :, 2 * D :])  # f32 -> bf16 cast

        # ---------------- Phase A: xT = transpose(x) -------------------
        if a_dmaT:
            # xbar transpose-DMA straight from the bf16 staging tile
            with tc.tile_pool(name="xload", bufs=3) as xpool:
                for so in range(SC):
                    x_t = xpool.tile([P, D], BF, tag="x")
                    nc.gpsimd.dma_start(x_t[:], x_ap[so * P : (so + 1) * P, :])
                    for dd in range(DC):
                        nc.sync.dma_start(
                            xT[:, dd, so * P : (so + 1) * P],
                            x_t[:, dd * P : (dd + 1) * P],
                            transpose=True,
                        )
        else:
            with tc.tile_pool(name="xload", bufs=3) as xpool, tc.tile_pool(
                name="pst", bufs=4, space="PSUM"
            ) as pst:
                for so in range(SC):
                    x_t = xpool.tile([P, D], BF, tag="x")
                    nc.gpsimd.dma_start(x_t[:], x_ap[so * P : (so + 1) * P, :])
                    for dd in range(DC):
                        ps = pst.tile([P, P], BF, tag="pt")
                        nc.tensor.transpose(
                            ps[:], x_t[:, dd * P : (dd + 1) * P], ident[:]
                        )
                        if dd % 2 == 0:
                            nc.scalar.copy(xT[:, dd, so * P : (so + 1) * P], ps[:])
                        else:
                            nc.vector.tensor_copy(
                                xT[:, dd, so * P : (so + 1) * P], ps[:]
                            )

        if phases == "a":
            with tc.tile_pool(name="dump", bufs=2) as dp:
                for dd in range(DC):
                    t = dp.tile([P, S], F32, tag="o", name=f"dump_{dd}")
                    nc.vector.tensor_copy(t[:], xT[:, dd, :])
                    nc.sync.dma_start(y_ap[dd * P : (dd + 1) * P, :], t[:])
            wvp.release()
            top.close()
            continue

        # ---------------- Phase B2: v = xT^T @ W_v (+ones) -------------
        ones_view = v_sb[:].rearrange("p so (h c) -> p so h c", c=65)[:, :, :, 64]
        nc.vector.tensor_copy(
            ones_view, nc.const_aps.tensor(1.0, list(ones_view.shape), F32)
        )
        p_b2 = tc.alloc_tile_pool(name="p_b2", bufs=1)
        biasv_bc = p_b2.tile([P, D], F32)
        nc.sync.dma_start(biasv_bc[:], bqkv_ap[2 * D :][None, :].to_broadcast((P, D)))
        with tc.tile_pool(name="psb2", bufs=6, space="PSUM") as psb2:
            for so in range(SC):
                pss2 = [
                    psb2.tile([P, NQ], F32, tag="ps2", name=f"ps2_{so}_{nq}")
                    for nq in range(D // NQ)
                ]
                if ldw_share:
                    # one LDWEIGHTS of xT[:, kc, so-chunk] serves both nq halves
                    for kc in range(DC):
                        for nq in range(D // NQ):
                            nc.tensor.matmul(
                                pss2[nq][:],
                                xT[:, kc, so * P : (so + 1) * P],
                                wv[:, kc, nq * NQ : (nq + 1) * NQ],
                                start=(kc == 0),
                                stop=(kc == DC - 1),
                            )
                for nq in range(D // NQ):
                    ps = pss2[nq]
                    if not ldw_share:
                        for kc in range(DC):
                            nc.tensor.matmul(
                                ps[:],
                                xT[:, kc, so * P : (so + 1) * P],
                                wv[:, kc, nq * NQ : (nq + 1) * NQ],
                                start=(kc == 0),
                                stop=(kc == DC - 1),
                            )
                    dest = v_sb[:, so, :].rearrange("p (h c) -> p h c", c=65)[
                        :, 8 * nq : 8 * nq + 8, 0:64
                    ]
                    nc.vector.tensor_tensor(
                        out=dest,
                        in0=ps[:].rearrange("p (h c) -> p h c", c=64),
                        in1=biasv_bc[:, nq * NQ : (nq + 1) * NQ].rearrange(
                            "p (h c) -> p h c", c=64
                        ),
                        op=mybir.AluOpType.add,
                    )
        p_b2.release()
        wvp.release()

        if phases == "ab":
            with tc.tile_pool(name="dump", bufs=2) as dp:
                for so in range(SC):
                    t = dp.tile([P, S], F32, tag="o", name=f"dump_{so}")
                    nc.vector.tensor_copy(t[:], v_sb[:, so, 0:S])
                    nc.sync.dma_start(y_ap[so * P : (so + 1) * P, :], t[:])
            top.close()
            continue

        # ---------------- Sweeps: B1 + attention + D --------------------
        wop = tc.alloc_tile_pool(name="wo", bufs=1)
        wo_half = []
        for half in range(2):
            woh = wop.tile([P, DC, NQ], BF, name=f"wo_{half}", tag=f"wo{half}")
            nc.gpsimd.dma_start(woh[:], wout_r[:, :, half * NQ : (half + 1) * NQ])
            wo_half.append(woh)

        cs = ExitStack()
        wqp = None if use_early_wq else cs.enter_context(
            tc.tile_pool(name="wq", bufs=wq_bufs)
        )
        psS = cs.enter_context(tc.tile_pool(name="psS", bufs=psS_bufs, space="PSUM"))
        psO = cs.enter_context(tc.tile_pool(name="psO", bufs=po_bufs, space="PSUM"))
        epool = cs.enter_context(tc.tile_pool(name="e", bufs=e_bufs))
        ogpool = cs.enter_context(tc.tile_pool(name="og", bufs=3))
        rtpool = cs.enter_context(tc.tile_pool(name="rt", bufs=2))
        drpool = cs.enter_context(tc.tile_pool(name="dr", bufs=2, space="DRAM"))
        bcpool = cs.enter_context(tc.tile_pool(name="bc", bufs=2))
        stpool = cs.enter_context(tc.tile_pool(name="st", bufs=2))
        # allocated last so it can be released (LIFO) before psY is created
        psb1 = tc.alloc_tile_pool(name="psb1", bufs=psb1_bufs, space="PSUM")

        def emit_B1(m):
            for mm in (m, 8 + m):
                if use_early_wq:
                    wq = wq_tiles[mm]
                else:
                    wq = wqp.tile([P, DC, P], BF, tag="wq", name=f"wq_{mm}")
                    nc.gpsimd.dma_start(wq[:], wqkv_r[:, :, mm * P : (mm + 1) * P])
                pss1 = [
                    psb1.tile([P, NQ], F32, tag="psb1", name=f"psb1_{mm}_{nq}")
                    for nq in range(S // NQ)
                ]
                if ldw_share:
                    for kc in range(DC):
                        for nq in range(S // NQ):
                            nc.tensor.matmul(
                                pss1[nq][:],
                                wq[:, kc, :],
                                xT[:, kc, nq * NQ : (nq + 1) * NQ],
                                start=(kc == 0),
                                stop=(kc == DC - 1),
                            )
                for nq in range(S // NQ):
                    ps = pss1[nq]
                    if not ldw_share:
                        for kc in range(DC):
                            nc.tensor.matmul(
                                ps[:],
                                wq[:, kc, :],
                                xT[:, kc, nq * NQ : (nq + 1) * NQ],
                                start=(kc == 0),
                                stop=(kc == DC - 1),
                            )
                    nc.vector.tensor_scalar(
                        out=qkT[:, mm, nq * NQ : (nq + 1) * NQ],
                        in0=ps[:],
                        scalar1=bqkv_sb[:, mm : mm + 1],
                        scalar2=None,
                        op0=mybir.AluOpType.add,
                    )

        def emit_C(m, j):
            nkc = 4 * (j + 1)
            poE = psO.tile([65, NQ], F32, tag="po", name=f"poE_{m}_{j}")
            poO = psO.tile([65, NQ], F32, tag="po", name=f"poO_{m}_{j}")
            for i in range(nkc):
                i_loc = i - 4 * j
                dead = 0 if uniform else max(0, i_loc * P)
                pss = psS.tile([P, 2, NQ], F32, tag="psS", name=f"psS_{m}_{j}_{i}")
                for idx, base in enumerate((0, 64)):
                    nc.tensor.matmul(
                        pss[:, idx, dead:],
                        qkT[base : base + 64, 8 + m, i * P : (i + 1) * P],
                        qkT[base : base + 64, m, j * NQ + dead : (j + 1) * NQ],
                        start=True,
                        stop=True,
                    )
                e = epool.tile([P, 2, NQ], BF, tag="e", name=f"e_{m}_{j}_{i}")
                nc.scalar.activation(
                    e[:, :, dead:],
                    pss[:, :, dead:],
                    mybir.ActivationFunctionType.Exp,
                    scale=0.125,
                )
                if i_loc >= 0:
                    # triangle-mask the 128-wide diagonal block: keep y' >= x
                    nc.gpsimd.affine_select(
                        out=e[:, :, dead : dead + P],
                        in_=e[:, :, dead : dead + P],
                        compare_op=mybir.AluOpType.is_ge,
                        fill=0.0,
                        base=0,
                        pattern=[[0, 2], [1, P]],
                        channel_multiplier=-1,
                    )
                for idx, h in enumerate((2 * m, 2 * m + 1)):
                    po = poE if idx == 0 else poO
                    nc.tensor.matmul(
                        po[:, dead:],
                        v_sb[:, i, 65 * h : 65 * h + 65],
                        e[:, idx, dead:],
                        start=(i == 0),
                        stop=(i == nkc - 1),
                    )
            for idx in range(2):
                po = poE if idx == 0 else poO
                if og_stage:
                    og = ogpool.tile([65, NQ], F32, tag="og", name=f"og_{m}_{j}_{idx}")
                    nc.vector.tensor_copy(og[:], po[:])
                else:
                    og = po  # normalize straight out of PSUM
                rt = rtpool.tile([1, NQ], F32, tag="rt", name=f"rt_{m}_{j}_{idx}")
                nc.vector.reciprocal(rt[:], og[64:65, :])
                rbc = bcpool.tile([64, NQ], F32, tag="rbc", name=f"rbc_{m}_{j}_{idx}")
                if pb_bcast:
                    nc.gpsimd.partition_broadcast(rbc[:], rt[:], channels=64)
                else:
                    scr = drpool.tile([NQ], F32, tag="scr", name=f"scr_{m}_{j}_{idx}")
                    nc.sync.dma_start(scr[None, :], rt[:])
                    nc.sync.dma_start(rbc[:], scr[None, :].to_broadcast((64, NQ)))
                if idx == 0:
                    nc.vector.tensor_tensor(
                        out=attnT[0:64, m, j * NQ : (j + 1) * NQ],
                        in0=og[0:64, :],
                        in1=rbc[:],
                        op=mybir.AluOpType.mult,
                    )
                else:
                    st = stpool.tile([64, NQ], BF, tag="st", name=f"st_{m}_{j}")
                    nc.vector.tensor_tensor(
                        out=st[:],
                        in0=og[0:64, :],
                        in1=rbc[:],
                        op=mybir.AluOpType.mult,
                    )
                    nc.sync.dma_start(
                        attnT[64:128, m, j * NQ : (j + 1) * NQ], st[:]
                    )

        # sweep 1: B1 interleaved with C(j=0)
        if phases == "b1only":
            for m in range(H // 2):
                emit_B1(m)
            psb1.release()
            with tc.tile_pool(name="dump", bufs=2) as dp:
                for mm in range(8):
                    t = dp.tile([P, S], F32, tag="o", name=f"dump_{mm}")
                    nc.vector.tensor_copy(t[:], qkT[:, mm, :])
                    nc.sync.dma_start(y_ap[mm * P : (mm + 1) * P, :], t[:])
            cs.close()
            wop.release()
            if use_early_wq:
                wqp0.release()
            top.close()
            continue
        for m in range(H // 2):
            emit_B1(m)
            emit_C(m, 0)
        psb1.release()

        if phases == "abs1":
            with tc.tile_pool(name="dump", bufs=2) as dp:
                for dd in range(DC):
                    t = dp.tile([P, NQ], F32, tag="o", name=f"dump_{dd}")
                    nc.vector.tensor_copy(t[:], attnT[:, dd, 0:NQ])
                    nc.sync.dma_start(y_ap[dd * P : (dd + 1) * P, 0:NQ], t[:])
            cs.close()
            wop.release()
            if use_early_wq:
                wqp0.release()
            top.close()
            continue

        # D setup
        psY = cs.enter_context(tc.tile_pool(name="psY", bufs=psY_bufs, space="PSUM"))
        ypool = cs.enter_context(tc.tile_pool(name="yp", bufs=3))
        boutbc = ypool.tile([P, D], F32, name="boutbc", tag="boutbc")
        nc.sync.dma_start(boutbc[:], bout_ap[None, :].to_broadcast((P, D)))

        def emit_D(qc):
            psy = [
                psY.tile([P, NQ], F32, tag="psY", name=f"psY_{qc}_{nqq}")
                for nqq in range(D // NQ)
            ]
            if ldw_share:
                for kc in range(DC):
                    for nqq in range(D // NQ):
                        nc.tensor.matmul(
                            psy[nqq][:],
                            attnT[:, kc, qc * P : (qc + 1) * P],
                            wo_half[nqq][:, kc, :],
                            start=(kc == 0),
                            stop=(kc == DC - 1),
                        )
            for nqq in range(D // NQ):
                ps = psy[nqq]
                if not ldw_share:
                    for kc in range(DC):
                        nc.tensor.matmul(
                            ps[:],
                            attnT[:, kc, qc * P : (qc + 1) * P],
                            wo_half[nqq][:, kc, :],
                            start=(kc == 0),
                            stop=(kc == DC - 1),
                        )
                yt = ypool.tile([P, NQ], F32, tag="y", name=f"y_{qc}_{nqq}")
                nc.vector.tensor_tensor(
                    out=yt[:],
                    in0=ps[:],
                    in1=boutbc[:, nqq * NQ : (nqq + 1) * NQ],
                    op=mybir.AluOpType.add,
                )
                nc.sync.dma_start(
                    y_ap[qc * P : (qc + 1) * P, nqq * NQ : (nqq + 1) * NQ], yt[:]
                )

        # sweep 2: C(j=1) with first-half D interleaved as PE filler
        nqc = 0
        for m in range(H // 2):
            emit_C(m, 1)
            if m in d_after:
                emit_D(nqc)
                nqc += 1
        for qc in range(nqc, SC):
            emit_D(qc)

        cs.close()
        wop.release()
        if use_early_wq:
            wqp0.release()
        top.close()

    nc.compile()
    return nc


_CACHED = {}


def _get_nc():
    if "nc" not in _CACHED:
        _CACHED["nc"] = build_kernel(niter=1)
    return _CACHED["nc"]


def kernel(x, W_qkv, b_qkv, W_out, b_out):
    x = np.ascontiguousarray(np.asarray(x, dtype=np.float32))
    W_qkv = np.ascontiguousarray(np.asarray(W_qkv, dtype=np.float32))
    b_qkv = np.ascontiguousarray(np.asarray(b_qkv, dtype=np.float32))
    W_out = np.ascontiguousarray(np.asarray(W_out, dtype=np.float32))
    b_out = np.ascontiguousarray(np.asarray(b_out, dtype=np.float32))
    B = x.shape[0]
    assert x.shape == (8, S, D), f"expected x [8, {S}, {D}], got {x.shape}"

    from concourse.bass_utils import run_bass_kernel_spmd

    nc = _get_nc()
    in_maps = [
        {
            "x": np.ascontiguousarray(x[b]),
            "W_qkv": W_qkv,
            "b_qkv": b_qkv,
            "W_out": W_out,
            "b_out": b_out,
        }
        for b in range(B)
    ]
    res = run_bass_kernel_spmd(nc, in_maps, list(range(B)))
    return np.stack([res.results[b]["y"] for b in range(B)]).astype(np.float32)

